# revision 5
# baseline (speedup 1.0000x reference)
"""Distributed 2-layer GCN (+mean-pool +MLP head) on 8 Trainium2 NeuronCores.

Sharding: 1-D node partition by dst. Each core owns a contiguous dst range and
all edges incident to it (edge lists sorted by dst tile, bank-grouped for the
layer-2 gather). Layer-1 source features are shipped pre-gathered in edge
order (the halo exchange materialized at input-sharding time); layer-2
gathers from the all-gathered g1 table on device via dma_gather.

Per dst tile of 128 nodes the scatter-reduce is a chain of one-hot matmuls on
the tensor engine: scT[ch, dst] += Xe_chunk[e, ch]^T @ onehot[e, dst], with
the GCN weight applied once per tile afterwards ((M^T Xe) W == M^T (Xe W)).
Everything on chip stays channel-major so no transposes are needed until the
per-tile epilogue.
"""

import math
import time
import numpy as np
import ml_dtypes

from concourse import bass, bacc, mybir, tile
from concourse.bass_utils import run_bass_kernel_spmd
from concourse.library_config import mlp
from concourse.masks import make_identity

BF16 = ml_dtypes.bfloat16
P = 128
NCORES = 8
GROUP = 8          # chunks per batched is_equal
BN_EPS = 1e-5


def _full_cfg():
    return dict(N=100000, CH=128, NG=128)


def _preprocess(x, edge_index, batch, W1, b1, gamma, beta, rmean, rvar,
                W2, b2, cfg):
    N, CH = cfg["N"], cfg["CH"]
    NDST = N // NCORES
    NT = math.ceil(NDST / P)
    NDST_PAD = NT * P
    NODES_PAD = NCORES * NDST_PAD
    assert NODES_PAD % 4 == 0
    BANK = NODES_PAD // 4
    assert BANK <= 32768 and BANK % P == 0

    src = np.asarray(edge_index[0], dtype=np.int64)
    dst = np.asarray(edge_index[1], dtype=np.int64)
    loop = np.arange(N, dtype=np.int64)
    src = np.concatenate([src, loop])
    dst = np.concatenate([dst, loop])

    deg = np.bincount(dst, minlength=N).astype(np.float64)
    dinv = (1.0 / np.sqrt(deg)).astype(np.float32)   # deg >= 1 (self loops)

    x_pre = (np.asarray(x, np.float32) * dinv[:, None]).astype(BF16)

    core = dst // NDST
    dloc = dst - core * NDST
    t_of = dloc // P
    rel = (dloc % P).astype(np.int64)
    # table row of a source node (shard layout with per-core padding)
    score = src // NDST
    trow = score * NDST_PAD + (src - score * NDST)
    bank = trow // BANK
    lidx = (trow - bank * BANK).astype(np.int64)

    # chunk quotas per (t, b), maxed over cores so the SPMD program is uniform
    key = ((core * NT + t_of) * 4 + bank)
    counts = np.bincount(key, minlength=NCORES * NT * 4).reshape(NCORES, NT, 4)
    q_tb = np.ceil(counts.max(axis=0) / P).astype(np.int64)      # [NT, 4]
    q_t = q_tb.sum(axis=1)                                       # [NT]
    coff = np.concatenate([[0], np.cumsum(q_t)])                 # [NT+1]
    TOTCH = int(coff[-1])
    # chunk offset of (t, b) relative to tile start
    cob = np.concatenate([np.zeros((NT, 1), np.int64),
                          np.cumsum(q_tb, axis=1)], axis=1)      # [NT, 5]

    # slot assignment per core
    order = np.argsort(key, kind="stable")
    gstart = np.concatenate([[0], np.cumsum(counts.reshape(-1))])
    within = np.empty(len(order), np.int64)
    within[order] = np.arange(len(order)) - gstart[key[order]]
    Cglob = (coff[t_of] + cob[t_of, bank] + within // P)         # global chunk
    pslot = within % P

    per_core = []
    for c in range(NCORES):
        m = core == c
        Ce, pe = Cglob[m], pslot[m]
        xe = np.zeros((P, TOTCH, CH), dtype=BF16)
        xe[pe, Ce] = x_pre[src[m]]
        relv = np.full((P, TOTCH), 255.0, dtype=BF16)
        relv[pe, Ce] = rel[m].astype(BF16)
        l16 = np.zeros((16, TOTCH * 8), dtype=np.int16)
        l16[pe % 16, Ce * 8 + pe // 16] = lidx[m].astype(np.int16)
        lall = np.tile(l16, (8, 1))
        bcol = np.full((P, NT), 255.0, dtype=BF16)
        nb = np.asarray(batch, np.int64)[c * NDST:(c + 1) * NDST]
        dl = np.arange(NDST)
        bcol[dl % P, dl // P] = nb.astype(BF16)
        dv = np.zeros((NDST_PAD,), np.float32)
        dv[:NDST] = dinv[c * NDST:(c + 1) * NDST]
        DINV = np.broadcast_to(dv[None, :], (P, NDST_PAD)).copy()
        per_core.append(dict(xe=xe, rel=relv, lidx=lall, bcol=bcol, DINV=DINV))

    S = (np.asarray(gamma, np.float32)
         / np.sqrt(np.asarray(rvar, np.float32) + BN_EPS))
    T = (np.asarray(beta, np.float32)
         - np.asarray(rmean, np.float32) * S
         + S * np.asarray(b1, np.float32))
    consts = dict(
        W1=np.asarray(W1, np.float32).astype(BF16),
        W2=np.asarray(W2, np.float32).astype(BF16),
        SCOL=S.reshape(CH, 1).copy(),
        TCOL=T.reshape(CH, 1).copy(),
        IOTA=np.broadcast_to(np.arange(P, dtype=BF16)[None, :], (P, P)).copy(),
    )
    dims = dict(NT=NT, NDST=NDST, NDST_PAD=NDST_PAD, NODES_PAD=NODES_PAD,
                BANK=BANK, TOTCH=TOTCH, CH=CH, NG=cfg["NG"],
                q_t=q_t.tolist(), coff=coff.tolist(),
                q_tb=q_tb.tolist(), cob=cob.tolist())
    return per_core, consts, dims


def _build(dims):
    NT, TOTCH, CH = dims["NT"], dims["TOTCH"], dims["CH"]
    NDST_PAD, NODES_PAD, BANK = dims["NDST_PAD"], dims["NODES_PAD"], dims["BANK"]
    q_t, coff, q_tb, cob = dims["q_t"], dims["coff"], dims["q_tb"], dims["cob"]
    qmax = max(q_t)
    bf = mybir.dt.bfloat16
    f32 = mybir.dt.float32

    nc = bacc.Bacc("TRN2", target_bir_lowering=False, debug=False,
                   enable_asserts=True, num_devices=NCORES,
                   num_swdge_queues=4)
    xe_p = nc.dram_tensor("xe", [P, TOTCH, CH], bf, kind="ExternalInput")
    rel_p = nc.dram_tensor("rel", [P, TOTCH], bf, kind="ExternalInput")
    lidx_p = nc.dram_tensor("lidx", [P, TOTCH * 8], mybir.dt.int16,
                            kind="ExternalInput")
    bcol_p = nc.dram_tensor("bcol", [P, NT], bf, kind="ExternalInput")
    dinv_p = nc.dram_tensor("DINV", [P, NDST_PAD], f32, kind="ExternalInput")
    w1_p = nc.dram_tensor("W1", [CH, CH], bf, kind="ExternalInput")
    w2_p = nc.dram_tensor("W2", [CH, CH], bf, kind="ExternalInput")
    scol_p = nc.dram_tensor("SCOL", [CH, 1], f32, kind="ExternalInput")
    tcol_p = nc.dram_tensor("TCOL", [CH, 1], f32, kind="ExternalInput")
    iota_p = nc.dram_tensor("IOTA", [P, P], bf, kind="ExternalInput")
    out_p = nc.dram_tensor("pooled", [P, CH], f32, kind="ExternalOutput")

    g1shard = nc.dram_tensor("g1shard", [NDST_PAD, CH], bf)
    g1table = nc.dram_tensor("g1table", [NODES_PAD, CH], bf)

    with tile.TileContext(nc) as tc:
        with tc.tile_critical():
            nc.gpsimd.load_library(mlp)
        with (
            tc.tile_pool(name="const", bufs=1) as cp,
            tc.tile_pool(name="xep", bufs=3) as xep,
            tc.tile_pool(name="idxp", bufs=3) as idxp,
            tc.tile_pool(name="mtp", bufs=4) as mtp,
            tc.tile_pool(name="sb", bufs=6) as sb,
            tc.tile_pool(name="psA", bufs=2, space="PSUM") as psA,
            tc.tile_pool(name="psB", bufs=2, space="PSUM") as psB,
            tc.tile_pool(name="psC", bufs=2, space="PSUM") as psC,
            tc.tile_pool(name="psPool", bufs=1, space="PSUM") as psPool,
        ):
            W1s = cp.tile([CH, CH], bf)
            nc.sync.dma_start(out=W1s[:], in_=w1_p[:, :])
            W2s = cp.tile([CH, CH], bf)
            nc.sync.dma_start(out=W2s[:], in_=w2_p[:, :])
            SCOLs = cp.tile([CH, 1], f32)
            nc.sync.dma_start(out=SCOLs[:], in_=scol_p[:, :])
            TCOLs = cp.tile([CH, 1], f32)
            nc.sync.dma_start(out=TCOLs[:], in_=tcol_p[:, :])
            IOTAs = cp.tile([P, P], bf)
            nc.sync.dma_start(out=IOTAs[:], in_=iota_p[:, :])
            DINVs = cp.tile([P, NDST_PAD], f32)
            nc.sync.dma_start(out=DINVs[:], in_=dinv_p[:, :])
            RELs = cp.tile([P, TOTCH], bf)
            nc.sync.dma_start(out=RELs[:], in_=rel_p[:, :])
            BCOLs = cp.tile([P, NT], bf)
            nc.sync.dma_start(out=BCOLs[:], in_=bcol_p[:, :])
            IDbf = cp.tile([P, P], bf)
            make_identity(nc, IDbf[:])

            def scatter_tile(t, blk):
                """one-hot matmul chain for tile t -> psum [ch, dst] f32"""
                q = q_t[t]
                scT = psA.tile([CH, P], f32)
                for g0 in range(0, q, GROUP):
                    nb = min(GROUP, q - g0)
                    MT = mtp.tile([P, GROUP, P], bf, tag="mt")
                    nc.vector.tensor_tensor(
                        out=MT[:, :nb, :],
                        in0=RELs[:, coff[t] + g0:coff[t] + g0 + nb, None]
                            .to_broadcast([P, nb, P]),
                        in1=IOTAs[:, None, :].to_broadcast([P, nb, P]),
                        op=mybir.AluOpType.is_equal,
                    )
                    for j in range(nb):
                        C = g0 + j
                        nc.tensor.matmul(
                            scT[:], lhsT=blk[:, C, :], rhs=MT[:, j, :],
                            start=(C == 0), stop=(C == q - 1),
                        )
                return scT

            def linear_epilogue(t, scT, Ws):
                """[ch_in, dst] psum -> W^T @ scT -> [ch_out, dst] psum"""
                scS = sb.tile([CH, P], bf, tag="scS")
                nc.any.tensor_copy(out=scS[:], in_=scT[:])
                p2 = psB.tile([CH, P], f32)
                nc.tensor.matmul(p2[:], lhsT=Ws[:], rhs=scS[:],
                                 start=True, stop=True)
                return p2

            # ---------------- layer 1 ----------------
            for t in range(NT):
                q = q_t[t]
                blk = xep.tile([P, qmax, CH], bf, tag="xe")
                nc.sync.dma_start(out=blk[:, :q, :],
                                  in_=xe_p[:, coff[t]:coff[t] + q, :])
                scT = scatter_tile(t, blk)
                p2 = linear_epilogue(t, scT, W1s)
                dsl = DINVs[:, t * P:(t + 1) * P]
                t2 = sb.tile([CH, P], f32, tag="t2")
                nc.vector.tensor_tensor(out=t2[:], in0=p2[:], in1=dsl,
                                        op=mybir.AluOpType.mult)
                h1 = sb.tile([CH, P], f32, tag="h1")
                nc.scalar.activation(h1[:], t2[:],
                                     mybir.ActivationFunctionType.Relu,
                                     bias=TCOLs[:], scale=SCOLs[:])
                g1c = sb.tile([CH, P], bf, tag="g1c")
                nc.vector.tensor_tensor(out=g1c[:], in0=h1[:], in1=dsl,
                                        op=mybir.AluOpType.mult)
                pt = psC.tile([P, CH], bf)
                nc.tensor.transpose(pt[:], g1c[:], IDbf[:])
                g1r = sb.tile([P, CH], bf, tag="g1r")
                nc.any.tensor_copy(out=g1r[:], in_=pt[:])
                nc.sync.dma_start(out=g1shard[t * P:(t + 1) * P, :],
                                  in_=g1r[:])

            # ---------------- all-gather ----------------
            nc.gpsimd.collective_compute(
                "AllGather", mybir.AluOpType.bypass,
                replica_groups=[list(range(NCORES))],
                ins=[g1shard.ap().opt()],
                outs=[g1table.ap().opt()],
            )

            # ---------------- layer 2 ----------------
            pooledP = psPool.tile([P, CH], f32)
            for t in range(NT):
                q = q_t[t]
                blk = xep.tile([P, qmax, CH], bf, tag="xe")
                lx = idxp.tile([P, qmax * 8], mybir.dt.int16, tag="lx")
                nc.sync.dma_start(out=lx[:, :q * 8],
                                  in_=lidx_p[:, coff[t] * 8:(coff[t] + q) * 8])
                for b in range(4):
                    qb = q_tb[t][b]
                    if qb == 0:
                        continue
                    co = cob[t][b]
                    nc.gpsimd.dma_gather(
                        blk[:, co:co + qb, :],
                        g1table[b * BANK:(b + 1) * BANK, :],
                        lx[:, co * 8:(co + qb) * 8],
                        qb * P, qb * P, CH,
                        single_packet=False, queue_num=b,
                    )
                scT = scatter_tile(t, blk)
                p2 = linear_epilogue(t, scT, W2s)
                dsl = DINVs[:, t * P:(t + 1) * P]
                h2c = sb.tile([CH, P], bf, tag="h2c")
                nc.vector.tensor_tensor(out=h2c[:], in0=p2[:], in1=dsl,
                                        op=mybir.AluOpType.mult)
                pt = psC.tile([P, CH], bf)
                nc.tensor.transpose(pt[:], h2c[:], IDbf[:])
                h2r = sb.tile([P, CH], bf, tag="h2r")
                nc.any.tensor_copy(out=h2r[:], in_=pt[:])
                OG = sb.tile([P, P], bf, tag="og")
                nc.vector.tensor_tensor(
                    out=OG[:],
                    in0=BCOLs[:, t:t + 1].to_broadcast([P, P]),
                    in1=IOTAs[:],
                    op=mybir.AluOpType.is_equal,
                )
                nc.tensor.matmul(pooledP[:], lhsT=OG[:], rhs=h2r[:],
                                 start=(t == 0), stop=(t == NT - 1))

            pooledS = sb.tile([P, CH], f32, tag="pooled")
            nc.any.tensor_copy(out=pooledS[:], in_=pooledP[:])
            nc.sync.dma_start(out=out_p[:, :], in_=pooledS[:])

    nc.finalize()
    return nc


_CACHE = {}


def _get_program(dims):
    key = (dims["NT"], dims["TOTCH"], tuple(dims["q_t"]),
           tuple(map(tuple, dims["q_tb"])))
    if key not in _CACHE:
        _CACHE[key] = _build(dims)
    return _CACHE[key]


def run(inputs, cfg, trace=False):
    tpp = time.time()
    per_core, consts, dims = _preprocess(
        inputs["x"], inputs["edge_index"], inputs["batch"], inputs["W1"],
        inputs["b1"], inputs["gamma"], inputs["beta"], inputs["rmean"],
        inputs["rvar"], inputs["W2"], inputs["b2"], cfg)
    t0 = time.time()
    nc = _get_program(dims)
    print(f"[kernel] build+finalize: {time.time()-t0:.1f}s  TOTCH={dims['TOTCH']} NT={dims['NT']}", flush=True)
    in_maps = []
    for c in range(NCORES):
        m = dict(per_core[c])
        m.update(consts)
        in_maps.append(m)
    t0 = time.time()
    res = run_bass_kernel_spmd(nc, in_maps, core_ids=list(range(NCORES)), trace=trace)
    print(f"[kernel] run: {time.time()-t0:.1f}s", flush=True)

    # host: cross-core reduce, mean, +b2, MLP head (tiny)
    NG = cfg["NG"]
    pooled = np.zeros((P, cfg["CH"]), np.float64)
    for c in range(NCORES):
        pooled += res.results[c]["pooled"].astype(np.float64)
    batch = np.asarray(inputs["batch"], np.int64)
    cnts = np.bincount(batch, minlength=NG).astype(np.float64)
    pooled = pooled[:NG] / np.maximum(cnts, 1.0)[:, None]
    pooled = pooled + np.asarray(inputs["b2"], np.float64)[None, :]
    z = pooled @ np.asarray(inputs["fw1"], np.float64)
    z = np.maximum(z + np.asarray(inputs["fb1"], np.float64), 0.0)
    out = z @ np.asarray(inputs["cw"], np.float64) \
        + np.asarray(inputs["cb"], np.float64)
    return out.astype(np.float32), res


def kernel(**inputs):
    out, _ = run(inputs, _full_cfg())
    return out


# revision 8
# speedup vs baseline: 1.2139x; 1.2139x over previous
"""Distributed 2-layer GCN (+mean-pool +MLP head) on 8 Trainium2 NeuronCores.

Sharding: 1-D node partition by dst. Each core owns a contiguous dst range and
all edges incident to it (edge lists sorted by dst tile, bank-grouped for the
layer-2 gather). Layer-1 source features are shipped pre-gathered in edge
order (the halo exchange materialized at input-sharding time); layer-2
gathers from the all-gathered g1 table on device via dma_gather.

Per dst tile of 128 nodes the scatter-reduce is a chain of one-hot matmuls on
the tensor engine: scT[ch, dst] += Xe_chunk[e, ch]^T @ onehot[e, dst], with
the GCN weight applied once per tile afterwards ((M^T Xe) W == M^T (Xe W)).
Everything on chip stays channel-major so no transposes are needed until the
per-tile epilogue.
"""

import math
import time
import numpy as np
import ml_dtypes

from concourse import bass, bacc, mybir, tile
from concourse.bass_utils import run_bass_kernel_spmd
from concourse.library_config import mlp
from concourse.masks import make_identity

BF16 = ml_dtypes.bfloat16
P = 128
NCORES = 8
GROUP = 8          # chunks per batched is_equal
BN_EPS = 1e-5


def _full_cfg():
    return dict(N=100000, CH=128, NG=128)


def _preprocess(x, edge_index, batch, W1, b1, gamma, beta, rmean, rvar,
                W2, b2, cfg):
    N, CH = cfg["N"], cfg["CH"]
    NDST = N // NCORES
    NT = math.ceil(NDST / P)
    NDST_PAD = NT * P
    NODES_PAD = NCORES * NDST_PAD
    # tile groups -> per-group all-gather banks for the layer-2 gather
    gsz = [NT // 4 + (1 if i < NT % 4 else 0) for i in range(4)]
    gstart = [sum(gsz[:i]) for i in range(5)]
    BANKSZ = [NCORES * g * P for g in gsz]
    assert all(b <= 32768 for b in BANKSZ)

    src = np.asarray(edge_index[0], dtype=np.int64)
    dst = np.asarray(edge_index[1], dtype=np.int64)
    loop = np.arange(N, dtype=np.int64)
    src = np.concatenate([src, loop])
    dst = np.concatenate([dst, loop])

    deg = np.bincount(dst, minlength=N).astype(np.float64)
    dinv = (1.0 / np.sqrt(deg)).astype(np.float32)   # deg >= 1 (self loops)

    x_pre = (np.asarray(x, np.float32) * dinv[:, None]).astype(BF16)

    core = dst // NDST
    dloc = dst - core * NDST
    t_of = dloc // P
    rel = (dloc % P).astype(np.int64)
    # bank = tile group of the source node; row within bank =
    # owner_core * group_rows + offset of the node inside its group
    score = src // NDST
    sloc = src - score * NDST
    st = sloc // P
    gs_arr = np.asarray(gstart)
    gsz_arr = np.asarray(gsz)
    grp_of_tile = np.searchsorted(gs_arr[1:], np.arange(NT), side="right")
    bank = grp_of_tile[st]
    lidx = (score * gsz_arr[bank] * P
            + (sloc - gs_arr[bank] * P)).astype(np.int64)

    # chunk quotas per (t, b), maxed over cores so the SPMD program is uniform
    key = ((core * NT + t_of) * 4 + bank)
    counts = np.bincount(key, minlength=NCORES * NT * 4).reshape(NCORES, NT, 4)
    q_tb = np.ceil(counts.max(axis=0) / P).astype(np.int64)      # [NT, 4]
    q_t = q_tb.sum(axis=1)                                       # [NT]
    coff = np.concatenate([[0], np.cumsum(q_t)])                 # [NT+1]
    TOTCH = int(coff[-1])
    # chunk offset of (t, b) relative to tile start
    cob = np.concatenate([np.zeros((NT, 1), np.int64),
                          np.cumsum(q_tb, axis=1)], axis=1)      # [NT, 5]

    # slot assignment per core
    order = np.argsort(key, kind="stable")
    kstart = np.concatenate([[0], np.cumsum(counts.reshape(-1))])
    within = np.empty(len(order), np.int64)
    within[order] = np.arange(len(order)) - kstart[key[order]]
    Cglob = (coff[t_of] + cob[t_of, bank] + within // P)         # global chunk
    pslot = within % P

    per_core = []
    for c in range(NCORES):
        m = core == c
        Ce, pe = Cglob[m], pslot[m]
        xe = np.zeros((P, TOTCH, CH), dtype=BF16)
        xe[pe, Ce] = x_pre[src[m]]
        relv = np.full((P, TOTCH), 255.0, dtype=BF16)
        relv[pe, Ce] = rel[m].astype(BF16)
        l16 = np.zeros((16, TOTCH * 8), dtype=np.int16)
        l16[pe % 16, Ce * 8 + pe // 16] = lidx[m].astype(np.int16)
        lall = np.tile(l16, (8, 1))
        bcol = np.full((P, NT), 255.0, dtype=BF16)
        nb = np.asarray(batch, np.int64)[c * NDST:(c + 1) * NDST]
        dl = np.arange(NDST)
        bcol[dl % P, dl // P] = nb.astype(BF16)
        dv = np.zeros((NDST_PAD,), np.float32)
        dv[:NDST] = dinv[c * NDST:(c + 1) * NDST]
        DINV = np.broadcast_to(dv[None, :], (P, NDST_PAD)).copy()
        per_core.append(dict(xe=xe, rel=relv, lidx=lall, bcol=bcol, DINV=DINV))

    S = (np.asarray(gamma, np.float32)
         / np.sqrt(np.asarray(rvar, np.float32) + BN_EPS))
    T = (np.asarray(beta, np.float32)
         - np.asarray(rmean, np.float32) * S
         + S * np.asarray(b1, np.float32))
    consts = dict(
        W1=np.asarray(W1, np.float32).astype(BF16),
        W2=np.asarray(W2, np.float32).astype(BF16),
        SCOL=S.reshape(CH, 1).copy(),
        TCOL=T.reshape(CH, 1).copy(),
        IOTA=np.broadcast_to(np.arange(P, dtype=BF16)[None, :], (P, P)).copy(),
    )
    dims = dict(NT=NT, NDST=NDST, NDST_PAD=NDST_PAD, NODES_PAD=NODES_PAD,
                GSZ=gsz, GSTART=gstart, BANKSZ=BANKSZ,
                TOTCH=TOTCH, CH=CH, NG=cfg["NG"],
                q_t=q_t.tolist(), coff=coff.tolist(),
                q_tb=q_tb.tolist(), cob=cob.tolist())
    return per_core, consts, dims


def _build(dims):
    NT, TOTCH, CH = dims["NT"], dims["TOTCH"], dims["CH"]
    NDST_PAD = dims["NDST_PAD"]
    GSZ, GSTART, BANKSZ = dims["GSZ"], dims["GSTART"], dims["BANKSZ"]
    q_t, coff, q_tb, cob = dims["q_t"], dims["coff"], dims["q_tb"], dims["cob"]
    qmax = max(q_t)
    bf = mybir.dt.bfloat16
    f32 = mybir.dt.float32

    nc = bacc.Bacc("TRN2", target_bir_lowering=False, debug=False,
                   enable_asserts=True, num_devices=NCORES,
                   num_swdge_queues=4)
    xe_p = nc.dram_tensor("xe", [P, TOTCH, CH], bf, kind="ExternalInput")
    rel_p = nc.dram_tensor("rel", [P, TOTCH], bf, kind="ExternalInput")
    lidx_p = nc.dram_tensor("lidx", [P, TOTCH * 8], mybir.dt.int16,
                            kind="ExternalInput")
    bcol_p = nc.dram_tensor("bcol", [P, NT], bf, kind="ExternalInput")
    dinv_p = nc.dram_tensor("DINV", [P, NDST_PAD], f32, kind="ExternalInput")
    w1_p = nc.dram_tensor("W1", [CH, CH], bf, kind="ExternalInput")
    w2_p = nc.dram_tensor("W2", [CH, CH], bf, kind="ExternalInput")
    scol_p = nc.dram_tensor("SCOL", [CH, 1], f32, kind="ExternalInput")
    tcol_p = nc.dram_tensor("TCOL", [CH, 1], f32, kind="ExternalInput")
    iota_p = nc.dram_tensor("IOTA", [P, P], bf, kind="ExternalInput")
    out_p = nc.dram_tensor("pooled", [P, CH], f32, kind="ExternalOutput")

    g1shard = nc.dram_tensor("g1shard", [NDST_PAD, CH], bf)
    g1banks = [nc.dram_tensor(f"g1bank{b}", [BANKSZ[b], CH], bf)
               if BANKSZ[b] > 0 else None for b in range(4)]

    with tile.TileContext(nc) as tc:
        with tc.tile_critical():
            nc.gpsimd.load_library(mlp)
        with (
            tc.tile_pool(name="const", bufs=1) as cp,
            tc.tile_pool(name="xep", bufs=3) as xep,
            tc.tile_pool(name="gatp", bufs=4) as gatp,
            tc.tile_pool(name="idxp", bufs=4) as idxp,
            tc.tile_pool(name="mtp", bufs=4) as mtp,
            tc.tile_pool(name="sb", bufs=6) as sb,
            tc.tile_pool(name="psA", bufs=2, space="PSUM") as psA,
            tc.tile_pool(name="psB", bufs=2, space="PSUM") as psB,
            tc.tile_pool(name="psC", bufs=2, space="PSUM") as psC,
            tc.tile_pool(name="psPool", bufs=1, space="PSUM") as psPool,
        ):
            W1s = cp.tile([CH, CH], bf)
            nc.sync.dma_start(out=W1s[:], in_=w1_p[:, :])
            W2s = cp.tile([CH, CH], bf)
            nc.sync.dma_start(out=W2s[:], in_=w2_p[:, :])
            SCOLs = cp.tile([CH, 1], f32)
            nc.sync.dma_start(out=SCOLs[:], in_=scol_p[:, :])
            TCOLs = cp.tile([CH, 1], f32)
            nc.sync.dma_start(out=TCOLs[:], in_=tcol_p[:, :])
            IOTAs = cp.tile([P, P], bf)
            nc.sync.dma_start(out=IOTAs[:], in_=iota_p[:, :])
            DINVs = cp.tile([P, NDST_PAD], f32)
            nc.sync.dma_start(out=DINVs[:], in_=dinv_p[:, :])
            RELs = cp.tile([P, TOTCH], bf)
            nc.sync.dma_start(out=RELs[:], in_=rel_p[:, :])
            BCOLs = cp.tile([P, NT], bf)
            nc.sync.dma_start(out=BCOLs[:], in_=bcol_p[:, :])
            IDbf = cp.tile([P, P], bf)
            make_identity(nc, IDbf[:])

            def scatter_tile(t, blk):
                """one-hot matmul chain for tile t -> psum [ch, dst] f32"""
                q = q_t[t]
                scT = psA.tile([CH, P], f32)
                for g0 in range(0, q, GROUP):
                    nb = min(GROUP, q - g0)
                    MT = mtp.tile([P, GROUP, P], bf, tag="mt")
                    nc.vector.tensor_tensor(
                        out=MT[:, :nb, :],
                        in0=RELs[:, coff[t] + g0:coff[t] + g0 + nb, None]
                            .to_broadcast([P, nb, P]),
                        in1=IOTAs[:, None, :].to_broadcast([P, nb, P]),
                        op=mybir.AluOpType.is_equal,
                    )
                    for j in range(nb):
                        C = g0 + j
                        nc.tensor.matmul(
                            scT[:], lhsT=blk[:, C, :], rhs=MT[:, j, :],
                            start=(C == 0), stop=(C == q - 1),
                        )
                return scT

            def linear_epilogue(t, scT, Ws):
                """[ch_in, dst] psum -> W^T @ scT -> [ch_out, dst] psum"""
                scS = sb.tile([CH, P], bf, tag="scS")
                nc.any.tensor_copy(out=scS[:], in_=scT[:])
                p2 = psB.tile([CH, P], f32)
                nc.tensor.matmul(p2[:], lhsT=Ws[:], rhs=scS[:],
                                 start=True, stop=True)
                return p2

            # ---------------- layer 1 ----------------
            for t in range(NT):
                q = q_t[t]
                blk = xep.tile([P, qmax, CH], bf, tag="xe")
                nc.sync.dma_start(out=blk[:, :q, :],
                                  in_=xe_p[:, coff[t]:coff[t] + q, :])
                scT = scatter_tile(t, blk)
                p2 = linear_epilogue(t, scT, W1s)
                dsl = DINVs[:, t * P:(t + 1) * P]
                t2 = sb.tile([CH, P], f32, tag="t2")
                nc.vector.tensor_tensor(out=t2[:], in0=p2[:], in1=dsl,
                                        op=mybir.AluOpType.mult)
                h1 = sb.tile([CH, P], f32, tag="h1")
                nc.scalar.activation(h1[:], t2[:],
                                     mybir.ActivationFunctionType.Relu,
                                     bias=TCOLs[:], scale=SCOLs[:])
                g1c = sb.tile([CH, P], bf, tag="g1c")
                nc.vector.tensor_tensor(out=g1c[:], in0=h1[:], in1=dsl,
                                        op=mybir.AluOpType.mult)
                pt = psC.tile([P, CH], bf)
                nc.tensor.transpose(pt[:], g1c[:], IDbf[:])
                g1r = sb.tile([P, CH], bf, tag="g1r")
                nc.any.tensor_copy(out=g1r[:], in_=pt[:])
                nc.sync.dma_start(out=g1shard[t * P:(t + 1) * P, :],
                                  in_=g1r[:])
                for b in range(4):
                    if GSZ[b] > 0 and t == GSTART[b + 1] - 1:
                        # group b complete -> exchange it while later
                        # groups are still computing
                        nc.gpsimd.collective_compute(
                            "AllGather", mybir.AluOpType.bypass,
                            replica_groups=[list(range(NCORES))],
                            ins=[g1shard[GSTART[b] * P:GSTART[b + 1] * P, :]
                                 .opt()],
                            outs=[g1banks[b].ap().opt()],
                        )

            # ---------------- layer 2 ----------------
            pooledP = psPool.tile([P, CH], f32)
            for t in range(NT):
                q = q_t[t]
                blk = gatp.tile([P, qmax, CH], bf, tag="gat")
                lx = idxp.tile([P, qmax * 8], mybir.dt.int16, tag="lx")
                nc.sync.dma_start(out=lx[:, :q * 8],
                                  in_=lidx_p[:, coff[t] * 8:(coff[t] + q) * 8])
                for b in range(4):
                    qb = q_tb[t][b]
                    if qb == 0:
                        continue
                    co = cob[t][b]
                    nc.gpsimd.dma_gather(
                        blk[:, co:co + qb, :],
                        g1banks[b][:, :],
                        lx[:, co * 8:(co + qb) * 8],
                        qb * P, qb * P, CH,
                        single_packet=False, queue_num=b,
                    )
                scT = scatter_tile(t, blk)
                p2 = linear_epilogue(t, scT, W2s)
                dsl = DINVs[:, t * P:(t + 1) * P]
                h2c = sb.tile([CH, P], bf, tag="h2c")
                nc.vector.tensor_tensor(out=h2c[:], in0=p2[:], in1=dsl,
                                        op=mybir.AluOpType.mult)
                pt = psC.tile([P, CH], bf)
                nc.tensor.transpose(pt[:], h2c[:], IDbf[:])
                h2r = sb.tile([P, CH], bf, tag="h2r")
                nc.any.tensor_copy(out=h2r[:], in_=pt[:])
                OG = sb.tile([P, P], bf, tag="og")
                nc.vector.tensor_tensor(
                    out=OG[:],
                    in0=BCOLs[:, t:t + 1].to_broadcast([P, P]),
                    in1=IOTAs[:],
                    op=mybir.AluOpType.is_equal,
                )
                nc.tensor.matmul(pooledP[:], lhsT=OG[:], rhs=h2r[:],
                                 start=(t == 0), stop=(t == NT - 1))

            pooledS = sb.tile([P, CH], f32, tag="pooled")
            nc.any.tensor_copy(out=pooledS[:], in_=pooledP[:])
            nc.sync.dma_start(out=out_p[:, :], in_=pooledS[:])

    nc.finalize()
    return nc


_CACHE = {}


def _get_program(dims):
    key = (dims["NT"], dims["TOTCH"], tuple(dims["q_t"]),
           tuple(map(tuple, dims["q_tb"])))
    if key not in _CACHE:
        _CACHE[key] = _build(dims)
    return _CACHE[key]


def run(inputs, cfg, trace=False):
    tpp = time.time()
    per_core, consts, dims = _preprocess(
        inputs["x"], inputs["edge_index"], inputs["batch"], inputs["W1"],
        inputs["b1"], inputs["gamma"], inputs["beta"], inputs["rmean"],
        inputs["rvar"], inputs["W2"], inputs["b2"], cfg)
    t0 = time.time()
    nc = _get_program(dims)
    print(f"[kernel] build+finalize: {time.time()-t0:.1f}s  TOTCH={dims['TOTCH']} NT={dims['NT']}", flush=True)
    in_maps = []
    for c in range(NCORES):
        m = dict(per_core[c])
        m.update(consts)
        in_maps.append(m)
    t0 = time.time()
    res = run_bass_kernel_spmd(nc, in_maps, core_ids=list(range(NCORES)), trace=trace)
    print(f"[kernel] run: {time.time()-t0:.1f}s", flush=True)

    # host: cross-core reduce, mean, +b2, MLP head (tiny)
    NG = cfg["NG"]
    pooled = np.zeros((P, cfg["CH"]), np.float64)
    for c in range(NCORES):
        pooled += res.results[c]["pooled"].astype(np.float64)
    batch = np.asarray(inputs["batch"], np.int64)
    cnts = np.bincount(batch, minlength=NG).astype(np.float64)
    pooled = pooled[:NG] / np.maximum(cnts, 1.0)[:, None]
    pooled = pooled + np.asarray(inputs["b2"], np.float64)[None, :]
    z = pooled @ np.asarray(inputs["fw1"], np.float64)
    z = np.maximum(z + np.asarray(inputs["fb1"], np.float64), 0.0)
    out = z @ np.asarray(inputs["cw"], np.float64) \
        + np.asarray(inputs["cb"], np.float64)
    return out.astype(np.float32), res


def kernel(**inputs):
    out, _ = run(inputs, _full_cfg())
    return out
